# revision 6
# baseline (speedup 1.0000x reference)
"""Trainium2 Bass kernel for nn_ControlledAttentionBlock (self-contained).

Data-parallel over batch: B=16 across 8 NeuronCores (2 batches/core).
All matmuls in float32r (fp32 memory, full PE rate, ~1e-4 rel err), loaded
straight from DRAM via bitcast APs (HW rounds on read).

Per core (batch-local L=512 tokens, D=1024, H=4 heads, hd=256):
  LN1 -> PE-transpose -> s_lnT [D, T=1024]; then per batch b:
    qT_b/kT_b [d_out, 512] (lhsT = w natural), v_b [512, d_out]
    per head: scoresT [m, l] psum -> col-0 collapse-bias fix -> exp ->
      rowsums (ones-matmul) -> recip -> PE row-broadcast -> normalize ae ->
      attn@v -> aoT[b] [d, l].  Head 0 redone in [l, m] layout -> attn0 out.
  out-proj + residual + bo -> s2 [T, D]; LN2 (g, b) -> transpose -> hT;
  FFN f-groups of 512 (mm2 psum accumulates over group PAIRS);
  s2 += b2 + ffn -> s_out.
"""
import numpy as np

import concourse.bass as bass
import concourse.tile as tile
from concourse import bacc, mybir
from concourse.bass_utils import run_bass_kernel_spmd
from concourse.masks import make_identity

F32 = mybir.dt.float32
F32R = mybir.dt.float32r
AF = mybir.ActivationFunctionType
ALU = mybir.AluOpType
AX = mybir.AxisListType

B, L, D = 16, 512, 1024
H, HD = 4, 256
NCORES = 8
BL = B // NCORES          # 2 batches per core
T = BL * L                # 1024 tokens per core
LT = T // 128             # 8 l-tiles per core
KC = D // 128             # 8 contraction chunks
F = 4 * D
EPS = 1e-5
SC = 1.0 / float(np.sqrt(HD))

_CACHE = {}


def _build():
    nc = bacc.Bacc("TRN2", target_bir_lowering=False, debug=False,
                   num_devices=NCORES)
    inp = {}
    for name, shape in [
        ("s", [BL, L, D]), ("wq", [D, D]), ("bq", [D]), ("wk", [D, D]),
        ("bk", [D]), ("wv", [D, D]), ("bv", [D]), ("wo", [D, D]), ("bo", [D]),
        ("cbias", [512]), ("sharp", [1]), ("ln_g", [D]), ("ln_b", [D]),
        ("w1", [D, F]), ("b1", [F]), ("w2", [F, D]), ("b2", [D]),
    ]:
        inp[name] = nc.dram_tensor(name, shape, F32, kind="ExternalInput").ap()
    s_out = nc.dram_tensor("s_out", [BL, L, D], F32, kind="ExternalOutput").ap()
    attn0 = nc.dram_tensor("attn0", [BL, L, L], F32, kind="ExternalOutput").ap()

    s_tiled = inp["s"].rearrange("b (c p) d -> (b c) p d", p=128)
    so_tiled = s_out.rearrange("b (c p) d -> (b c) p d", p=128)
    a0_tiled = attn0.rearrange("b (c p) m -> (b c) p m", p=128)

    def wview(w):  # [D_in, n] fp32 dram -> [128, ko, n] f32r view
        return w.bitcast(F32R).rearrange("(ko ki) n -> ki ko n", ki=128)

    def bcast(ap, p=128):  # prepend a stride-0 partition dim
        return bass.AP(tensor=ap.tensor, offset=ap.offset, ap=[[0, p]] + list(ap.ap))

    with tile.TileContext(nc) as tc:
        with (
            tc.tile_pool(name="wstream", bufs=2) as wpool,
            tc.tile_pool(name="misc", bufs=1) as misc,
            tc.tile_pool(name="stage", bufs=2) as stage,
            tc.tile_pool(name="small", bufs=4) as smallp,
            tc.tile_pool(name="vrow", bufs=3) as vrowp,
            tc.tile_pool(name="aop", bufs=2) as aop,
            tc.tile_pool(name="ps", bufs=4, space="PSUM") as psp,
            tc.tile_pool(name="ps_t", bufs=2, space="PSUM") as pstp,
            tc.tile_pool(name="ps_row", bufs=2, space="PSUM") as psrp,
        ):
            # ---- constants / biases ----
            ident = misc.tile([128, 128], F32)
            make_identity(nc, ident)
            ones_col_f = misc.tile([128, 1], F32)
            nc.vector.memset(ones_col_f, 1.0)
            ones_col = misc.tile([128, 1], F32R)
            nc.vector.tensor_copy(ones_col, ones_col_f)
            ones_row = misc.tile([1, 128], F32R)
            nc.vector.tensor_copy(ones_row,
                                  ones_col_f[0:1, 0:1].to_broadcast((1, 128)))
            eps_t = misc.tile([128, 1], F32)
            nc.vector.memset(eps_t, EPS)

            def col_bias(name, n):
                t = misc.tile([128, n], F32, name=f"cb_{name}")
                nc.sync.dma_start(t, inp[name].rearrange("(ko ki) -> ki ko", ki=128))
                return t

            bq_sb = col_bias("bq", KC)
            bk_sb = col_bias("bk", KC)
            b1_sb = col_bias("b1", F // 128)

            def row_bias(name):
                t = misc.tile([128, D], F32, name=f"rb_{name}")
                nc.sync.dma_start(t, bcast(inp[name]))
                return t

            bv_bc = row_bias("bv")
            bo_bc = row_bias("bo")
            b2_bc = row_bias("b2")
            g_bc = row_bias("ln_g")
            gb_bc = row_bias("ln_b")
            cb_col = misc.tile([128, 4], F32)
            nc.sync.dma_start(cb_col,
                              inp["cbias"].rearrange("(ko ki) -> ki ko", ki=128))
            cb_row = misc.tile([1, 512], F32)
            nc.sync.dma_start(cb_row, bcast(inp["cbias"], p=1))
            sharp_col = misc.tile([128, 1], F32)
            nc.sync.dma_start(sharp_col, bcast(inp["sharp"]))

            def ln_stats(x):
                st = smallp.tile([128, 2, 6], F32, name="bnst")
                nc.vector.bn_stats(st[:, 0, :], x[:, 0:512])
                nc.vector.bn_stats(st[:, 1, :], x[:, 512:1024])
                mv = smallp.tile([128, 2], F32, name="bnmv")
                nc.vector.bn_aggr(mv, st)
                rstd = smallp.tile([128, 1], F32, name="rstd")
                nc.scalar.activation(rstd, mv[:, 1:2], AF.Sqrt, bias=eps_t,
                                     scale=1.0)
                nc.vector.reciprocal(rstd, rstd)
                return mv[:, 0:1], rstd

            def transpose_to(dst_col_fn, src_tile):
                for j in range(KC):
                    pt = pstp.tile([128, 128], F32, name="pt")
                    nc.tensor.transpose(pt, src_tile[:, j * 128:(j + 1) * 128],
                                        ident)
                    nc.scalar.copy(dst_col_fn(j), pt)

            aoT = [aop.tile([128, KC, 512], F32R, name="aoT") for _ in range(BL)]

            with tc.tile_pool(name="slnT", bufs=1) as slnp:
                # ---- LN1 + transpose ----
                s_lnT = slnp.tile([128, KC, T], F32R)
                for lt in range(LT):
                    srow = stage.tile([128, D], F32, name="st4k")
                    nc.sync.dma_start(srow, s_tiled[lt])
                    mean, rstd = ln_stats(srow)
                    sln = stage.tile([128, D], F32, name="st4k")
                    nc.vector.tensor_scalar(out=sln, in0=srow, scalar1=mean,
                                            scalar2=rstd, op0=ALU.subtract,
                                            op1=ALU.mult)
                    transpose_to(
                        lambda j, lt=lt: s_lnT[:, j, lt * 128:(lt + 1) * 128],
                        sln)

                with tc.tile_pool(name="qkv", bufs=3) as qkvp:
                    for b in range(BL):
                        boff = b * 512
                        # ---- QKV for batch b ----
                        qT = qkvp.tile([128, KC, 512], F32R, name="qkvb")
                        kT = qkvp.tile([128, KC, 512], F32R, name="qkvb")
                        v_sb = qkvp.tile([128, 4, D], F32R, name="qkvb")
                        for w_ap, bcol, dst in ((inp["wq"], bq_sb, qT),
                                                (inp["wk"], bk_sb, kT)):
                            wv_ = wview(w_ap)
                            for half in range(2):
                                wh = wpool.tile([128, KC, 512], F32R, name="wst")
                                nc.sync.dma_start(
                                    wh, wv_[:, :, half * 512:(half + 1) * 512])
                                for oc4 in range(4):
                                    oc = half * 4 + oc4
                                    ps = psp.tile([128, 512], F32, name="ps")
                                    for k in range(KC):
                                        nc.tensor.matmul(
                                            ps,
                                            wh[:, k, oc4 * 128:(oc4 + 1) * 128],
                                            s_lnT[:, k, boff:boff + 512],
                                            start=(k == 0), stop=(k == KC - 1))
                                    nc.scalar.activation(
                                        dst[:, oc, :], ps, AF.Identity,
                                        bias=bcol[:, oc:oc + 1], scale=1.0)
                        wv_v = wview(inp["wv"])
                        for dh in range(2):
                            wh = wpool.tile([128, KC, 512], F32R, name="wst")
                            nc.sync.dma_start(
                                wh, wv_v[:, :, dh * 512:(dh + 1) * 512])
                            for mt in range(4):
                                ps = psp.tile([128, 512], F32, name="ps")
                                for k in range(KC):
                                    nc.tensor.matmul(
                                        ps,
                                        s_lnT[:, k, boff + mt * 128:boff + (mt + 1) * 128],
                                        wh[:, k, :],
                                        start=(k == 0), stop=(k == KC - 1))
                                nc.vector.tensor_add(
                                    v_sb[:, mt, dh * 512:(dh + 1) * 512], ps,
                                    bv_bc[:, dh * 512:(dh + 1) * 512])

                        # ---- attention for batch b ----
                        with tc.tile_pool(name="aeP", bufs=2) as aep:
                            for h in range(H):
                                dc0 = 2 * h
                                ae = aep.tile([128, 4, 512], F32R, name="ae")
                                for mt in range(4):
                                    ps = psp.tile([128, 512], F32, name="ps")
                                    for i, dc in enumerate((dc0, dc0 + 1)):
                                        nc.tensor.matmul(
                                            ps,
                                            kT[:, dc, mt * 128:(mt + 1) * 128],
                                            qT[:, dc, :],
                                            start=(i == 0), stop=(i == 1))
                                    nc.scalar.activation(ae[:, mt, :], ps,
                                                         AF.Exp, scale=SC)
                                    tmp = smallp.tile([128, 1], F32, name="c0")
                                    nc.vector.tensor_scalar_mul(
                                        tmp, ps[:, 0:1], SC)
                                    nc.vector.tensor_add(
                                        tmp, tmp, cb_col[:, mt:mt + 1])
                                    nc.vector.tensor_scalar_mul(
                                        tmp, tmp, sharp_col)
                                    nc.scalar.activation(ae[:, mt, 0:1], tmp,
                                                         AF.Exp, scale=1.0)
                                psr = psrp.tile([1, 512], F32, name="psr")
                                for mt in range(4):
                                    nc.tensor.matmul(psr, ones_col,
                                                     ae[:, mt, :],
                                                     start=(mt == 0),
                                                     stop=(mt == 3))
                                rec_f = vrowp.tile([1, 512], F32, name="v512")
                                nc.vector.reciprocal(rec_f, psr)
                                rec = vrowp.tile([1, 512], F32R, name="v512")
                                nc.vector.tensor_copy(rec, rec_f)
                                psb = psp.tile([128, 512], F32, name="ps")
                                nc.tensor.matmul(psb, ones_row, rec,
                                                 start=True, stop=True)
                                for mt in range(4):
                                    nc.vector.tensor_mul(ae[:, mt, :],
                                                         ae[:, mt, :], psb)
                                for dt in range(2):
                                    ps = psp.tile([128, 512], F32, name="ps")
                                    for mt in range(4):
                                        nc.tensor.matmul(
                                            ps,
                                            v_sb[:, mt, h * HD + dt * 128:h * HD + (dt + 1) * 128],
                                            ae[:, mt, :],
                                            start=(mt == 0), stop=(mt == 3))
                                    nc.scalar.copy(aoT[b][:, dc0 + dt, :], ps)
                            # head-0 in [l, m] layout -> attn0 output
                            for lt4 in range(4):
                                ps = psp.tile([128, 512], F32, name="ps")
                                for i in range(2):
                                    nc.tensor.matmul(
                                        ps, qT[:, i, lt4 * 128:(lt4 + 1) * 128],
                                        kT[:, i, :],
                                        start=(i == 0), stop=(i == 1))
                                a0t = stage.tile([128, 512], F32, name="a0t")
                                nc.scalar.activation(a0t, ps, AF.Exp, scale=SC)
                                if lt4 == 0:
                                    tr = vrowp.tile([1, 512], F32, name="v512")
                                    nc.vector.tensor_scalar_mul(tr, ps[0:1, :], SC)
                                    nc.vector.tensor_add(tr, tr, cb_row)
                                    nc.vector.tensor_scalar_mul(
                                        tr, tr, sharp_col[0:1, 0:1])
                                    nc.scalar.activation(a0t[0:1, :], tr,
                                                         AF.Exp, scale=1.0)
                                rsum = smallp.tile([128, 1], F32, name="rsum")
                                nc.vector.reduce_sum(rsum, a0t, axis=AX.X)
                                nc.vector.reciprocal(rsum, rsum)
                                nc.vector.tensor_scalar_mul(a0t, a0t, rsum)
                                nc.sync.dma_start(a0_tiled[b * 4 + lt4], a0t)

            # ---- out-proj + residual -> s2 ----
            with tc.tile_pool(name="s2p", bufs=1) as s2p:
                s2 = s2p.tile([128, LT, D], F32)
                wo_v = wview(inp["wo"])
                woh = []
                for eh in range(2):
                    wt = wpool.tile([128, KC, 512], F32R, name="wst")
                    nc.sync.dma_start(wt, wo_v[:, :, eh * 512:(eh + 1) * 512])
                    woh.append(wt)
                for lt in range(LT):
                    b, mt = lt // 4, lt % 4
                    spb = stage.tile([128, D], F32, name="st4k")
                    nc.sync.dma_start(spb, s_tiled[lt])
                    nc.vector.tensor_add(spb, spb, bo_bc)
                    for eh in range(2):
                        ps = psp.tile([128, 512], F32, name="ps")
                        for dc in range(KC):
                            nc.tensor.matmul(
                                ps, aoT[b][:, dc, mt * 128:(mt + 1) * 128],
                                woh[eh][:, dc, :],
                                start=(dc == 0), stop=(dc == KC - 1))
                        nc.vector.tensor_add(
                            s2[:, lt, eh * 512:(eh + 1) * 512], ps,
                            spb[:, eh * 512:(eh + 1) * 512])

                # ---- LN2 + transpose -> hT ----
                with tc.tile_pool(name="hTp", bufs=1) as hTp:
                    hT = hTp.tile([128, KC, T], F32R)
                    for lt in range(LT):
                        mean, rstd = ln_stats(s2[:, lt, :])
                        h_t = stage.tile([128, D], F32, name="st4k")
                        nc.vector.tensor_scalar(out=h_t, in0=s2[:, lt, :],
                                                scalar1=mean, scalar2=rstd,
                                                op0=ALU.subtract, op1=ALU.mult)
                        nc.vector.tensor_mul(h_t, h_t, g_bc)
                        nc.vector.tensor_add(h_t, h_t, gb_bc)
                        transpose_to(
                            lambda j, lt=lt: hT[:, j, lt * 128:(lt + 1) * 128],
                            h_t)
                        nc.vector.tensor_add(s2[:, lt, :], s2[:, lt, :], b2_bc)

                    # ---- FFN: f-groups of 512, mm2 over group pairs ----
                    with tc.tile_pool(name="midp", bufs=2) as midp:
                        w1_v = wview(inp["w1"])
                        w2_v = inp["w2"].bitcast(F32R).rearrange(
                            "(ko ki) e -> ki ko e", ki=128)  # [128, 32, 1024]
                        NG = F // 512  # 8
                        mids = []
                        for g in range(NG):
                            w1g = wpool.tile([128, KC, 512], F32R, name="wst")
                            nc.sync.dma_start(
                                w1g, w1_v[:, :, g * 512:(g + 1) * 512])
                            mid = midp.tile([128, 4, T], F32R, name="mid")
                            for fc in range(4):
                                for th in range(2):
                                    ps = psp.tile([128, 512], F32, name="ps")
                                    for ec in range(KC):
                                        nc.tensor.matmul(
                                            ps,
                                            w1g[:, ec, fc * 128:(fc + 1) * 128],
                                            hT[:, ec, th * 512:(th + 1) * 512],
                                            start=(ec == 0),
                                            stop=(ec == KC - 1))
                                    nc.scalar.activation(
                                        mid[:, fc, th * 512:(th + 1) * 512],
                                        ps, AF.Relu,
                                        bias=b1_sb[:, g * 4 + fc:g * 4 + fc + 1],
                                        scale=1.0)
                            mids.append(mid)
                            if g % 2 == 1:
                                gp = g // 2
                                for eh in range(2):
                                    w2g = wpool.tile([128, KC, 512], F32R,
                                                     name="wst")
                                    nc.sync.dma_start(
                                        w2g,
                                        w2_v[:, gp * 8:(gp + 1) * 8,
                                             eh * 512:(eh + 1) * 512])
                                    for lt in range(LT):
                                        ps = psp.tile([128, 512], F32,
                                                      name="ps")
                                        for kk in range(KC):
                                            mid_k = mids[kk // 4]
                                            nc.tensor.matmul(
                                                ps,
                                                mid_k[:, kk % 4,
                                                      lt * 128:(lt + 1) * 128],
                                                w2g[:, kk, :],
                                                start=(kk == 0),
                                                stop=(kk == KC - 1))
                                        nc.vector.tensor_add(
                                            s2[:, lt, eh * 512:(eh + 1) * 512],
                                            s2[:, lt, eh * 512:(eh + 1) * 512],
                                            ps)
                                mids = []

                # ---- write out ----
                for lt in range(LT):
                    nc.sync.dma_start(so_tiled[lt], s2[:, lt, :])

    nc.compile()
    return nc


def _get_nc():
    if "nc" not in _CACHE:
        _CACHE["nc"] = _build()
    return _CACHE["nc"]


def kernel(**inputs):
    ins = {k: np.ascontiguousarray(np.asarray(v, dtype=np.float32))
           for k, v in inputs.items()}
    z = np.asarray(inputs["z"])
    nc = _get_nc()
    names = ["wq", "bq", "wk", "bk", "wv", "bv", "wo", "bo", "cbias", "sharp",
             "ln_g", "ln_b", "w1", "b1", "w2", "b2"]
    in_maps = []
    for c in range(NCORES):
        m = {"s": ins["s"][c * BL:(c + 1) * BL]}
        for n in names:
            m[n] = ins[n]
        in_maps.append(m)
    res = run_bass_kernel_spmd(nc, in_maps, core_ids=list(range(NCORES)))
    s_out = np.concatenate([r["s_out"] for r in res.results], axis=0)
    attn0 = np.concatenate([r["attn0"] for r in res.results], axis=0)
    return (s_out, z, attn0)


# revision 10
# speedup vs baseline: 1.0645x; 1.0645x over previous
"""Trainium2 Bass kernel for nn_ControlledAttentionBlock (self-contained).

Data-parallel over batch: B=16 across 8 NeuronCores (2 batches/core).
All matmuls in float32r (fp32 memory, full PE rate, ~1e-4 rel err), loaded
straight from DRAM via bitcast APs (HW rounds on read).

Per core (batch-local L=512 tokens, D=1024, H=4 heads, hd=256):
  LN1 -> PE-transpose -> s_lnT [D, T=1024]; then per batch b:
    qT_b/kT_b [d_out, 512] (lhsT = w natural), v_b [512, d_out]
    per head: scoresT [m, l] psum -> col-0 collapse-bias fix -> exp ->
      rowsums (ones-matmul) -> recip -> PE row-broadcast -> normalize ae ->
      attn@v -> aoT[b] [d, l].  Head 0 redone in [l, m] layout -> attn0 out.
  out-proj + residual + bo -> s2 [T, D]; LN2 (g, b) -> transpose -> hT;
  FFN f-groups of 512 (mm2 psum accumulates over group PAIRS);
  s2 += b2 + ffn -> s_out.
"""
import numpy as np

import concourse.bass as bass
import concourse.tile as tile
from concourse import bacc, mybir
from concourse.bass_utils import run_bass_kernel_spmd
from concourse.masks import make_identity

F32 = mybir.dt.float32
F32R = mybir.dt.float32r
AF = mybir.ActivationFunctionType
ALU = mybir.AluOpType
AX = mybir.AxisListType

B, L, D = 16, 512, 1024
H, HD = 4, 256
NCORES = 8
BL = B // NCORES          # 2 batches per core
T = BL * L                # 1024 tokens per core
LT = T // 128             # 8 l-tiles per core
KC = D // 128             # 8 contraction chunks
F = 4 * D
EPS = 1e-5
SC = 1.0 / float(np.sqrt(HD))

_CACHE = {}


def _build():
    nc = bacc.Bacc("TRN2", target_bir_lowering=False, debug=False,
                   num_devices=NCORES)
    inp = {}
    for name, shape in [
        ("s", [BL, L, D]), ("wq", [D, D]), ("bq", [D]), ("wk", [D, D]),
        ("bk", [D]), ("wv", [D, D]), ("bv", [D]), ("wo", [D, D]), ("bo", [D]),
        ("cbias", [512]), ("sharp", [1]), ("ln_g", [D]), ("ln_b", [D]),
        ("w1", [D, F]), ("b1", [F]), ("w2", [F, D]), ("b2", [D]),
    ]:
        inp[name] = nc.dram_tensor(name, shape, F32, kind="ExternalInput").ap()
    s_out = nc.dram_tensor("s_out", [BL, L, D], F32, kind="ExternalOutput").ap()
    attn0 = nc.dram_tensor("attn0", [BL, L, L], F32, kind="ExternalOutput").ap()

    s_tiled = inp["s"].rearrange("b (c p) d -> (b c) p d", p=128)
    so_tiled = s_out.rearrange("b (c p) d -> (b c) p d", p=128)
    a0_tiled = attn0.rearrange("b (c p) m -> (b c) p m", p=128)

    def wview(w):  # [D_in, n] fp32 dram -> [128, ko, n] f32r view
        return w.bitcast(F32R).rearrange("(ko ki) n -> ki ko n", ki=128)

    def bcast(ap, p=128):  # prepend a stride-0 partition dim
        return bass.AP(tensor=ap.tensor, offset=ap.offset, ap=[[0, p]] + list(ap.ap))

    with tile.TileContext(nc) as tc:
        with (
            tc.tile_pool(name="wstream", bufs=2) as wpool,
            tc.tile_pool(name="misc", bufs=1) as misc,
            tc.tile_pool(name="stage", bufs=3) as stage,
            tc.tile_pool(name="small", bufs=4) as smallp,
            tc.tile_pool(name="vrow", bufs=2) as vrowp,
            tc.tile_pool(name="aop", bufs=2) as aop,
            tc.tile_pool(name="ps", bufs=4, space="PSUM") as psp,
            tc.tile_pool(name="ps_t", bufs=2, space="PSUM") as pstp,
            tc.tile_pool(name="ps_row", bufs=2, space="PSUM") as psrp,
        ):
            # ---- constants / biases ----
            ident = misc.tile([128, 128], F32)
            make_identity(nc, ident)
            ones_col_f = misc.tile([128, 1], F32)
            nc.vector.memset(ones_col_f, 1.0)
            ones_col = misc.tile([128, 1], F32R)
            nc.vector.tensor_copy(ones_col, ones_col_f)
            ones_row = misc.tile([1, 128], F32R)
            nc.vector.tensor_copy(ones_row,
                                  ones_col_f[0:1, 0:1].to_broadcast((1, 128)))
            eps_t = misc.tile([128, 1], F32)
            nc.vector.memset(eps_t, EPS)

            def col_bias(name, n):
                t = misc.tile([128, n], F32, name=f"cb_{name}")
                nc.sync.dma_start(t, inp[name].rearrange("(ko ki) -> ki ko", ki=128))
                return t

            bq_sb = col_bias("bq", KC)
            bk_sb = col_bias("bk", KC)
            b1_sb = col_bias("b1", F // 128)

            def row_bias(name):
                t = misc.tile([128, D], F32, name=f"rb_{name}")
                nc.sync.dma_start(t, bcast(inp[name]))
                return t

            bv_bc = row_bias("bv")
            bo_bc = row_bias("bo")
            b2_bc = row_bias("b2")
            g_bc = row_bias("ln_g")
            gb_bc = row_bias("ln_b")
            cb_col = misc.tile([128, 4], F32)
            nc.sync.dma_start(cb_col,
                              inp["cbias"].rearrange("(ko ki) -> ki ko", ki=128))
            cb_row = misc.tile([1, 512], F32)
            nc.sync.dma_start(cb_row, bcast(inp["cbias"], p=1))
            sharp_col = misc.tile([128, 1], F32)
            nc.sync.dma_start(sharp_col, bcast(inp["sharp"]))

            def ln_stats(x):
                st = smallp.tile([128, 2, 6], F32, name="bnst")
                nc.vector.bn_stats(st[:, 0, :], x[:, 0:512])
                nc.vector.bn_stats(st[:, 1, :], x[:, 512:1024])
                mv = smallp.tile([128, 2], F32, name="bnmv")
                nc.vector.bn_aggr(mv, st)
                rstd = smallp.tile([128, 1], F32, name="rstd")
                nc.scalar.activation(rstd, mv[:, 1:2], AF.Sqrt, bias=eps_t,
                                     scale=1.0)
                nc.vector.reciprocal(rstd, rstd)
                return mv[:, 0:1], rstd

            def transpose_to(dst_col_fn, src_tile):
                for j in range(KC):
                    pt = pstp.tile([128, 128], F32, name="pt")
                    nc.tensor.transpose(pt, src_tile[:, j * 128:(j + 1) * 128],
                                        ident)
                    nc.scalar.copy(dst_col_fn(j), pt)

            aoT = [aop.tile([128, KC, 512], F32R, name="aoT") for _ in range(BL)]

            with tc.tile_pool(name="slnT", bufs=1) as slnp:
                # ---- LN1 + transpose ----
                s_lnT = slnp.tile([128, KC, T], F32R)
                for lt in range(LT):
                    srow = stage.tile([128, D], F32, name="st4k")
                    nc.sync.dma_start(srow, s_tiled[lt])
                    mean, rstd = ln_stats(srow)
                    sln = stage.tile([128, D], F32, name="st4k")
                    nc.vector.tensor_scalar(out=sln, in0=srow, scalar1=mean,
                                            scalar2=rstd, op0=ALU.subtract,
                                            op1=ALU.mult)
                    transpose_to(
                        lambda j, lt=lt: s_lnT[:, j, lt * 128:(lt + 1) * 128],
                        sln)

                with tc.tile_pool(name="qkv", bufs=3) as qkvp:
                    for b in range(BL):
                        boff = b * 512
                        # ---- QKV for batch b ----
                        qT = qkvp.tile([128, KC, 512], F32R, name="qkvb")
                        kT = qkvp.tile([128, KC, 512], F32R, name="qkvb")
                        v_sb = qkvp.tile([128, 4, D], F32R, name="qkvb")
                        for w_ap, bcol, dst in ((inp["wq"], bq_sb, qT),
                                                (inp["wk"], bk_sb, kT)):
                            wv_ = wview(w_ap)
                            for half in range(2):
                                wh = wpool.tile([128, KC, 512], F32R, name="wst")
                                nc.sync.dma_start(
                                    wh, wv_[:, :, half * 512:(half + 1) * 512])
                                for oc4 in range(4):
                                    oc = half * 4 + oc4
                                    ps = psp.tile([128, 512], F32, name="ps")
                                    for k in range(KC):
                                        nc.tensor.matmul(
                                            ps,
                                            wh[:, k, oc4 * 128:(oc4 + 1) * 128],
                                            s_lnT[:, k, boff:boff + 512],
                                            start=(k == 0), stop=(k == KC - 1))
                                    nc.scalar.activation(
                                        dst[:, oc, :], ps, AF.Identity,
                                        bias=bcol[:, oc:oc + 1], scale=1.0)
                        wv_v = wview(inp["wv"])
                        for dh in range(2):
                            wh = wpool.tile([128, KC, 512], F32R, name="wst")
                            nc.sync.dma_start(
                                wh, wv_v[:, :, dh * 512:(dh + 1) * 512])
                            for mt in range(4):
                                ps = psp.tile([128, 512], F32, name="ps")
                                for k in range(KC):
                                    nc.tensor.matmul(
                                        ps,
                                        s_lnT[:, k, boff + mt * 128:boff + (mt + 1) * 128],
                                        wh[:, k, :],
                                        start=(k == 0), stop=(k == KC - 1))
                                nc.vector.tensor_add(
                                    v_sb[:, mt, dh * 512:(dh + 1) * 512], ps,
                                    bv_bc[:, dh * 512:(dh + 1) * 512])

                        # ---- attention for batch b ----
                        with tc.tile_pool(name="aeP", bufs=2) as aep:
                            for h in range(H):
                                dc0 = 2 * h
                                ae = aep.tile([128, 4, 512], F32R, name="ae")
                                for mt in range(4):
                                    ps = psp.tile([128, 512], F32, name="ps")
                                    for i, dc in enumerate((dc0, dc0 + 1)):
                                        nc.tensor.matmul(
                                            ps,
                                            kT[:, dc, mt * 128:(mt + 1) * 128],
                                            qT[:, dc, :],
                                            start=(i == 0), stop=(i == 1))
                                    nc.scalar.activation(ae[:, mt, :], ps,
                                                         AF.Exp, scale=SC)
                                    tmp = smallp.tile([128, 1], F32, name="c0")
                                    nc.vector.tensor_scalar_mul(
                                        tmp, ps[:, 0:1], SC)
                                    nc.vector.tensor_add(
                                        tmp, tmp, cb_col[:, mt:mt + 1])
                                    nc.vector.tensor_scalar_mul(
                                        tmp, tmp, sharp_col)
                                    nc.scalar.activation(ae[:, mt, 0:1], tmp,
                                                         AF.Exp, scale=1.0)
                                psr = psrp.tile([1, 512], F32, name="psr")
                                for mt in range(4):
                                    nc.tensor.matmul(psr, ones_col,
                                                     ae[:, mt, :],
                                                     start=(mt == 0),
                                                     stop=(mt == 3))
                                pv = []
                                for dt in range(2):
                                    p_ = psp.tile([128, 512], F32, name="ps")
                                    for mt in range(4):
                                        nc.tensor.matmul(
                                            p_,
                                            v_sb[:, mt, h * HD + dt * 128:h * HD + (dt + 1) * 128],
                                            ae[:, mt, :],
                                            start=(mt == 0), stop=(mt == 3))
                                    pv.append(p_)
                                rec_f = vrowp.tile([1, 512], F32, name="v512")
                                nc.vector.reciprocal(rec_f, psr)
                                rec = vrowp.tile([1, 512], F32R, name="v512")
                                nc.vector.tensor_copy(rec, rec_f)
                                psb = psp.tile([128, 512], F32, name="ps")
                                nc.tensor.matmul(psb, ones_row, rec,
                                                 start=True, stop=True)
                                bc_sb = stage.tile([128, 512], F32, name="a0t")
                                nc.scalar.copy(bc_sb, psb)
                                for dt in range(2):
                                    nc.vector.tensor_mul(aoT[b][:, dc0 + dt, :],
                                                         pv[dt], bc_sb)
                            # head-0 in [l, m] layout -> attn0 output
                            for lt4 in range(4):
                                ps = psp.tile([128, 512], F32, name="ps")
                                for i in range(2):
                                    nc.tensor.matmul(
                                        ps, qT[:, i, lt4 * 128:(lt4 + 1) * 128],
                                        kT[:, i, :],
                                        start=(i == 0), stop=(i == 1))
                                a0t = stage.tile([128, 512], F32, name="a0t")
                                nc.scalar.activation(a0t, ps, AF.Exp, scale=SC)
                                if lt4 == 0:
                                    tr = vrowp.tile([1, 512], F32, name="v512")
                                    nc.vector.tensor_scalar_mul(tr, ps[0:1, :], SC)
                                    nc.vector.tensor_add(tr, tr, cb_row)
                                    nc.vector.tensor_scalar_mul(
                                        tr, tr, sharp_col[0:1, 0:1])
                                    nc.scalar.activation(a0t[0:1, :], tr,
                                                         AF.Exp, scale=1.0)
                                rsum = smallp.tile([128, 1], F32, name="rsum")
                                nc.vector.reduce_sum(rsum, a0t, axis=AX.X)
                                nc.vector.reciprocal(rsum, rsum)
                                nc.vector.tensor_scalar_mul(a0t, a0t, rsum)
                                nc.sync.dma_start(a0_tiled[b * 4 + lt4], a0t)

            # ---- out-proj + residual -> s2 ----
            with tc.tile_pool(name="s2p", bufs=1) as s2p:
                s2 = s2p.tile([128, LT, D], F32)
                wo_v = wview(inp["wo"])
                woh = []
                for eh in range(2):
                    wt = wpool.tile([128, KC, 512], F32R, name="wst")
                    nc.sync.dma_start(wt, wo_v[:, :, eh * 512:(eh + 1) * 512])
                    woh.append(wt)
                for lt in range(LT):
                    b, mt = lt // 4, lt % 4
                    spb = stage.tile([128, D], F32, name="st4k")
                    nc.sync.dma_start(spb, s_tiled[lt])
                    nc.vector.tensor_add(spb, spb, bo_bc)
                    for eh in range(2):
                        ps = psp.tile([128, 512], F32, name="ps")
                        for dc in range(KC):
                            nc.tensor.matmul(
                                ps, aoT[b][:, dc, mt * 128:(mt + 1) * 128],
                                woh[eh][:, dc, :],
                                start=(dc == 0), stop=(dc == KC - 1))
                        nc.vector.tensor_add(
                            s2[:, lt, eh * 512:(eh + 1) * 512], ps,
                            spb[:, eh * 512:(eh + 1) * 512])

                # ---- LN2 + transpose -> hT ----
                with tc.tile_pool(name="hTp", bufs=1) as hTp:
                    hT = hTp.tile([128, KC, T], F32R)
                    for lt in range(LT):
                        mean, rstd = ln_stats(s2[:, lt, :])
                        h_t = stage.tile([128, D], F32, name="st4k")
                        nc.vector.tensor_scalar(out=h_t, in0=s2[:, lt, :],
                                                scalar1=mean, scalar2=rstd,
                                                op0=ALU.subtract, op1=ALU.mult)
                        nc.vector.tensor_mul(h_t, h_t, g_bc)
                        nc.vector.tensor_add(h_t, h_t, gb_bc)
                        transpose_to(
                            lambda j, lt=lt: hT[:, j, lt * 128:(lt + 1) * 128],
                            h_t)
                        nc.vector.tensor_add(s2[:, lt, :], s2[:, lt, :], b2_bc)

                    # ---- FFN: f-groups of 512, mm2 over group pairs ----
                    with tc.tile_pool(name="midp", bufs=2) as midp:
                        w1_v = wview(inp["w1"])
                        w2_v = inp["w2"].bitcast(F32R).rearrange(
                            "(ko ki) e -> ki ko e", ki=128)  # [128, 32, 1024]
                        NG = F // 512  # 8
                        mids = []
                        for g in range(NG):
                            w1g = wpool.tile([128, KC, 512], F32R, name="wst")
                            nc.sync.dma_start(
                                w1g, w1_v[:, :, g * 512:(g + 1) * 512])
                            mid = midp.tile([128, 4, T], F32R, name="mid")
                            for fc in range(4):
                                for th in range(2):
                                    ps = psp.tile([128, 512], F32, name="ps")
                                    for ec in range(KC):
                                        nc.tensor.matmul(
                                            ps,
                                            w1g[:, ec, fc * 128:(fc + 1) * 128],
                                            hT[:, ec, th * 512:(th + 1) * 512],
                                            start=(ec == 0),
                                            stop=(ec == KC - 1))
                                    nc.scalar.activation(
                                        mid[:, fc, th * 512:(th + 1) * 512],
                                        ps, AF.Relu,
                                        bias=b1_sb[:, g * 4 + fc:g * 4 + fc + 1],
                                        scale=1.0)
                            mids.append(mid)
                            if g % 2 == 1:
                                gp = g // 2
                                for eh in range(2):
                                    w2g = wpool.tile([128, KC, 512], F32R,
                                                     name="wst")
                                    nc.sync.dma_start(
                                        w2g,
                                        w2_v[:, gp * 8:(gp + 1) * 8,
                                             eh * 512:(eh + 1) * 512])
                                    for lt in range(LT):
                                        ps = psp.tile([128, 512], F32,
                                                      name="ps")
                                        for kk in range(KC):
                                            mid_k = mids[kk // 4]
                                            nc.tensor.matmul(
                                                ps,
                                                mid_k[:, kk % 4,
                                                      lt * 128:(lt + 1) * 128],
                                                w2g[:, kk, :],
                                                start=(kk == 0),
                                                stop=(kk == KC - 1))
                                        nc.vector.tensor_add(
                                            s2[:, lt, eh * 512:(eh + 1) * 512],
                                            s2[:, lt, eh * 512:(eh + 1) * 512],
                                            ps)
                                mids = []

                # ---- write out ----
                for lt in range(LT):
                    nc.sync.dma_start(so_tiled[lt], s2[:, lt, :])

    nc.compile()
    return nc


def _get_nc():
    if "nc" not in _CACHE:
        _CACHE["nc"] = _build()
    return _CACHE["nc"]


def kernel(**inputs):
    ins = {k: np.ascontiguousarray(np.asarray(v, dtype=np.float32))
           for k, v in inputs.items()}
    z = np.asarray(inputs["z"])
    nc = _get_nc()
    names = ["wq", "bq", "wk", "bk", "wv", "bv", "wo", "bo", "cbias", "sharp",
             "ln_g", "ln_b", "w1", "b1", "w2", "b2"]
    in_maps = []
    for c in range(NCORES):
        m = {"s": ins["s"][c * BL:(c + 1) * BL]}
        for n in names:
            m[n] = ins[n]
        in_maps.append(m)
    res = run_bass_kernel_spmd(nc, in_maps, core_ids=list(range(NCORES)))
    s_out = np.concatenate([r["s_out"] for r in res.results], axis=0)
    attn0 = np.concatenate([r["attn0"] for r in res.results], axis=0)
    return (s_out, z, attn0)
